# revision 31
# baseline (speedup 1.0000x reference)
# Multi-head attention (b=2, n=2048, d_model=1024, 16 heads) on 8 NeuronCores.
#
# Sharding: core c = (batch b, head-group g) with b = c//4, g = c%4.
# Each core handles 1 batch element and 4 heads (256 channels), computing a
# partial output projection; the host sums the 4 group-partials per batch and
# adds b_O.
#
# v2 design (scalar-exp-bound schedule, ~147us EXP floor):
#  - Heads processed in PAIRS (cs in {0,1}; rows 0:64 / 64:128 of qt/kt[cs]).
#    The two score matmuls of a pair have K=64 and auto-derive PE row-tiles
#    (0,0)/(64,0) from their base partitions -> they stream CONCURRENTLY.
#  - Query chunks of 512; st pair-packed [128, 1024] (h_even | h_odd) in PSUM,
#    double-buffered; ONE [128,1024] Exp per (pair, m-slice) on ScalarE with
#    the 1/8 scale folded in, output DIRECTLY in fp8e4.
#  - A*V runs in fp8 DoubleRow: Ko=2 packs consecutive m-slices, so each
#    matmul streams 2 slices worth of E (half the PE stream time of bf16).
#    V is stored fp8 as v4p[mp] = [128, (ko=2, h=4, 72)] with a ones column
#    at offset 64 (softmax denominators fall out of PSUM row 64 for free).
#    (fp8 on E/V measured 1.7e-2 rel err vs the 2e-2 gate in host sim;
#    projections/scores stay bf16 - fp8 there blows the budget.)
#  - Segments run PAIR-MAJOR (all 4 chunks of pair 0, then pair 1) so kt[1]
#    isn't needed until slot 64. Q/K/V/O projection chains are deadline-paced
#    fillers eating PE idle under the scalar-bound attention loop; the et ring
#    (8 groups) lets A*V lag fillers without stalling ScalarE.

import ml_dtypes
import numpy as np

import concourse.bass as bass
import concourse.bacc as bacc
import concourse.tile as tile
from concourse import mybir
from concourse.bass_utils import run_bass_kernel_spmd

D = 1024  # d_model
N = 2048  # sequence length
B = 2  # batch
NHEADS = 16
DK = 64
NCORES = 8
GROUPS = 4  # head-groups across cores
HPG = NHEADS // GROUPS  # 4 heads per group
CH = HPG * DK  # 256 channels per group
KT = D // 128  # 8 contraction tiles for the projections
MS = N // 128  # 16 m-slices (key dim)
MP = MS // 2  # 8 m-slice pairs (DoubleRow Ko=2)
NCHUNK = 512  # query-chunk width
NCHUNKS = N // NCHUNK
VPITCH = 72  # per-head pitch in v4p (65 used, pad so ko-stride % 16 == 0)

F32 = mybir.dt.float32
F16 = mybir.dt.float16
BF16 = mybir.dt.bfloat16
FP8 = mybir.dt.float8e4


def _build_bass():
    nc = bacc.Bacc()

    xT_d = nc.dram_tensor("xT", [D, N], BF16, kind="ExternalInput")
    wqT_d = nc.dram_tensor("wqT", [D, CH], BF16, kind="ExternalInput")
    wkT_d = nc.dram_tensor("wkT", [D, CH], BF16, kind="ExternalInput")
    wvT_d = nc.dram_tensor("wvT", [D, CH], BF16, kind="ExternalInput")
    woT_d = nc.dram_tensor("woT", [CH, D], BF16, kind="ExternalInput")
    bq_d = nc.dram_tensor("bq", [CH], F32, kind="ExternalInput")
    bk_d = nc.dram_tensor("bk", [CH], F32, kind="ExternalInput")
    bv_d = nc.dram_tensor("bv", [CH], F32, kind="ExternalInput")
    # f16 output (10 mantissa bits, ~4x tighter than bf16; values << f16
    # range): halves the 8MB/core output DMA, which is descriptor-rate bound.
    yT_d = nc.dram_tensor("yT", [D, N], F16, kind="ExternalOutput")

    with tile.TileContext(nc) as tc:
        with (
            tc.tile_pool(name="persist", bufs=1) as persist,
            tc.tile_pool(name="et_pool", bufs=8) as et_pool,
            tc.tile_pool(name="osb_pool", bufs=1) as osb_pool,
            tc.tile_pool(name="small", bufs=2) as small,
            tc.tile_pool(name="aux_ps", bufs=2, space="PSUM") as aux_ps,
            tc.tile_pool(name="st_ps", bufs=2, space="PSUM") as st_pool,
            tc.tile_pool(name="ot_ps", bufs=1, space="PSUM") as ot_pool,
        ):
            # ---- input loads. Descriptor GENERATION (~1-2us per dma_start
            # on the issuing sequencer) was the bottleneck, not bandwidth:
            # weights load as ONE combined tile each via a 3D AP, the ones
            # column is a memset (no DMA), and bulk loads issue from the
            # otherwise-idle GpSimd sequencer so Sync stays free.
            xt = []
            wkall = persist.tile([128, KT * CH], BF16, tag="wkall", name="wkall")
            _b = wkT_d[:, :]
            wk_ap = bass.AP(tensor=_b.tensor, offset=_b.offset,
                            ap=[[CH, 128], [128 * CH, KT], [1, CH]])
            nc.gpsimd.dma_start(out=wkall.rearrange("p (k c) -> p k c", k=KT), in_=wk_ap)
            for k in range(KT):
                t = persist.tile([128, N], BF16, tag=f"xt{k}", name=f"xt{k}")
                xt.append(t)
                nc.gpsimd.dma_start(out=t[:, 0:512], in_=xT_d[k * 128 : (k + 1) * 128, 0:512])
            bball = persist.tile([128, 4], F32, tag="bball", name="bball")
            for i, dram in ((0, bq_d), (1, bk_d)):
                _b = dram[:]
                b_ap = bass.AP(tensor=_b.tensor, offset=_b.offset,
                               ap=[[1, 128], [128, 2]])
                nc.gpsimd.dma_start(out=bball[:, 2 * i : 2 * i + 2], in_=b_ap)
            bq_t = [bball[:, 0:1], bball[:, 1:2]]
            bk_t = [bball[:, 2:3], bball[:, 3:4]]
            wqall = persist.tile([128, KT * CH], BF16, tag="wqall", name="wqall")
            _b = wqT_d[:, :]
            wq_ap = bass.AP(tensor=_b.tensor, offset=_b.offset,
                            ap=[[CH, 128], [128 * CH, KT], [1, CH]])
            nc.gpsimd.dma_start(out=wqall.rearrange("p (k c) -> p k c", k=KT), in_=wq_ap)
            for k in range(KT):  # xT column block 1
                nc.gpsimd.dma_start(out=xt[k][:, 512:1024], in_=xT_d[k * 128 : (k + 1) * 128, 512:1024])
            wvall = persist.tile([128, KT * CH], BF16, tag="wvall", name="wvall")
            _b = wvT_d[:, :]
            wv_ap = bass.AP(tensor=_b.tensor, offset=_b.offset,
                            ap=[[CH, 128], [128 * CH, KT], [1, CH]])
            nc.gpsimd.dma_start(out=wvall.rearrange("p (k c) -> p k c", k=KT), in_=wv_ap)
            for k in range(KT):
                nc.gpsimd.dma_start(out=xt[k][:, 1024:2048], in_=xT_d[k * 128 : (k + 1) * 128, 1024:2048])
            wq = [wqall[:, k * CH : (k + 1) * CH] for k in range(KT)]
            wk = [wkall[:, k * CH : (k + 1) * CH] for k in range(KT)]
            wv = [wvall[:, k * CH : (k + 1) * CH] for k in range(KT)]
            bvb = persist.tile([128, CH], F32, tag="bvb", name="bvb")
            bv_ap = bv_d[None, :]
            nc.gpsimd.dma_start(
                out=bvb,
                in_=bass.AP(tensor=bv_ap.tensor, offset=bv_ap.offset, ap=[[0, 128]] + list(bv_ap.ap[1:])),
            )
            # ---- persistent tensors ----
            qt = [persist.tile([128, N], BF16, tag=f"qt{cs}", name=f"qt{cs}") for cs in range(CH // 128)]
            kt = [persist.tile([128, N], BF16, tag=f"kt{cs}", name=f"kt{cs}") for cs in range(CH // 128)]
            # v4p[mp]: fp8, layout [128, (ko=2, h=4, VPITCH)]; per head cols
            # h*VPITCH .. +64 = V channels, col 64 = ones (denominator trick)
            v4p = [persist.tile([128, 2 * HPG * VPITCH], FP8, tag=f"v4p{mp}", name=f"v4p{mp}") for mp in range(MP)]
            wotall = persist.tile([128, 2 * D], BF16, tag="wotall", name="wotall")
            _b = woT_d[:, :]
            wot_ap = bass.AP(tensor=_b.tensor, offset=_b.offset,
                             ap=[[D, 128], [128 * D, 2], [1, D]])
            nc.gpsimd.dma_start(out=wotall.rearrange("p (c d) -> p c d", c=2), in_=wot_ap)
            wot = [wotall[:, 0:D], wotall[:, D : 2 * D]]
            osb = {}
            for c in range(NCHUNKS):
                for cs in range(CH // 128):
                    osb[(c, cs)] = osb_pool.tile(
                        [128, NCHUNK], BF16, tag=f"osb{c}_{cs}", name=f"osb{c}_{cs}"
                    )

            # ---- filler emitters (projection chains on aux PSUM) ----
            def emit_v(ms):
                mp, ko = divmod(ms, 2)
                ps = aux_ps.tile([128, 512], F32, tag="aux", name="aux_ps_t")
                for k in range(KT):
                    nc.tensor.matmul(
                        ps[:, 0:CH],
                        xt[k][:, ms * 128 : (ms + 1) * 128],
                        wv[k],
                        start=(k == 0),
                        stop=(k == KT - 1),
                    )
                v4v = v4p[mp].rearrange("p (k h s) -> p k h s", k=2, h=HPG)
                if ko == 0:
                    nc.gpsimd.memset(v4v[:, :, :, 64:65], 1.0)
                nc.vector.tensor_add(
                    out=v4v[:, ko, :, 0:64],
                    in0=ps[:, 0:CH].rearrange("p (h c) -> p h c", c=64),
                    in1=bvb.rearrange("p (h c) -> p h c", c=64),
                )

            def emit_qk_chain(isq, cs, n0):
                dst, w, bias = (qt, wq, bq_t) if isq else (kt, wk, bk_t)
                ps = aux_ps.tile([128, 512], F32, tag="aux", name="aux_ps_t")
                for k in range(KT):
                    nc.tensor.matmul(
                        ps,
                        w[k][:, cs * 128 : (cs + 1) * 128],
                        xt[k][:, n0 : n0 + 512],
                        start=(k == 0),
                        stop=(k == KT - 1),
                    )
                nc.vector.tensor_scalar_add(
                    out=dst[cs][:, n0 : n0 + 512], in0=ps, scalar1=bias[cs]
                )

            def emit_f(c, msl):
                yp = aux_ps.tile([128, 512], F32, tag="aux", name="aux_yt_t")
                for cs in range(CH // 128):
                    nc.tensor.matmul(
                        yp,
                        wot[cs][:, msl * 128 : (msl + 1) * 128],
                        osb[(c, cs)],
                        start=(cs == 0),
                        stop=(cs == CH // 128 - 1),
                    )
                ysb = small.tile([128, 512], F16, tag="ysb", name="ysb_t", bufs=4)
                nc.vector.tensor_copy(out=ysb, in_=yp)
                nc.sync.dma_start(
                    out=yT_d[msl * 128 : (msl + 1) * 128, c * NCHUNK : (c + 1) * NCHUNK],
                    in_=ysb,
                )

            # chunk 3's output projection, split so only the cs1 half (which
            # needs seg 7's osb) lands in the epilogue: the cs0 half runs in
            # seg 4 and parks in SBUF.
            f3a_sb = {}

            def emit_f3a(msl):
                yp = aux_ps.tile([128, 512], F32, tag="aux", name="aux_yt_t")
                nc.tensor.matmul(
                    yp, wot[0][:, msl * 128 : (msl + 1) * 128], osb[(3, 0)],
                    start=True, stop=True,
                )
                t = small.tile([128, 512], F32, tag=f"f3a{msl}", name=f"f3a{msl}", bufs=1)
                nc.vector.tensor_copy(out=t, in_=yp)
                f3a_sb[msl] = t

            def emit_f3b(msl):
                yp = aux_ps.tile([128, 512], F32, tag="aux", name="aux_yt_t")
                nc.tensor.matmul(
                    yp, wot[1][:, msl * 128 : (msl + 1) * 128], osb[(3, 1)],
                    start=True, stop=True,
                )
                ysb = small.tile([128, 512], F16, tag="ysb", name="ysb_t", bufs=4)
                nc.vector.tensor_add(out=ysb, in0=yp, in1=f3a_sb[msl])
                nc.sync.dma_start(
                    out=yT_d[msl * 128 : (msl + 1) * 128, 3 * NCHUNK : 4 * NCHUNK],
                    in_=ysb,
                )

            # ---- prelude: just what segment 0 needs to start ----
            emit_qk_chain(False, 0, 0)  # kt[0] cols 0:512 (ms 0..3)
            emit_qk_chain(True, 0, 0)   # qt[0] cols 0:512 (chunk 0)

            # ---- deadline-paced fillers, emitted AFTER scores+exp of their
            # slot (so score matmuls always lead in PE queue order) and
            # BEFORE the A*V matmul of their m-slice pair (so v4p[mp] writes
            # precede the DoubleRow matmul that reads them - Tile derives
            # dependencies from program order).
            def V(ms):
                return lambda: emit_v(ms)

            def QK(isq, cs, n0):
                return lambda: emit_qk_chain(isq, cs, n0)

            def F(c, msl):
                return lambda: emit_f(c, msl)

            # fillers keyed by slot (2*mp+mi): emitted right after that
            # slot's exp (ScalarE rides the queued exp out) and before the
            # pair's A*V (v4p[mp] write-before-read ordering). One chain per
            # slot keeps the worst score-matmul delay ~1 chain.
            seg_fill = {
                0: {
                    0: [V(0)], 1: [V(1)], 2: [V(2)],
                    3: [V(3), QK(False, 0, 512)],
                    4: [V(4)],
                    5: [V(5), QK(True, 0, 512)],
                    6: [V(6)],
                    7: [V(7), QK(False, 0, 1024)],
                    8: [V(8)], 9: [V(9)], 10: [V(10)],
                    11: [V(11), QK(False, 0, 1536)],
                    12: [V(12)], 13: [V(13)], 14: [V(14)], 15: [V(15)],
                },
                1: {
                    1: [QK(True, 0, 1024)],
                    3: [QK(False, 1, 0)],
                    5: [QK(False, 1, 512)],
                    7: [QK(True, 0, 1536)],
                    9: [QK(False, 1, 1024)],
                    11: [QK(False, 1, 1536)],
                    13: [QK(True, 1, 0)],
                },
                2: {
                    1: [QK(True, 1, 512)],
                    5: [QK(True, 1, 1024)],
                    9: [QK(True, 1, 1536)],
                },
                3: {},
                4: {2 * msl: [lambda msl=msl: emit_f3a(msl)] for msl in range(D // 128)},
                5: {2 * msl: [lambda msl=msl: emit_f(0, msl)] for msl in range(D // 128)},
                6: {2 * msl: [lambda msl=msl: emit_f(1, msl)] for msl in range(D // 128)},
                7: {2 * msl: [lambda msl=msl: emit_f(2, msl)] for msl in range(D // 128)},
            }

            # ---- attention: pair-major segments ----
            seg = 0
            deferred = []
            for cs in range(2):
                for c in range(NCHUNKS):
                    n0 = c * NCHUNK
                    fillers = seg_fill[seg]
                    ot = [
                        ot_pool.tile([65, NCHUNK], F32, tag=f"ot{hi}", name=f"ot{hi}_t")
                        for hi in range(2)
                    ]
                    for mp in range(MP):
                        et = et_pool.tile([128, 2048], FP8, tag="et", name="et_t")
                        for mi in range(2):
                            ms = 2 * mp + mi
                            st = st_pool.tile([128, 1024], F32, tag="st", name="st_t")
                            for hi in range(2):
                                r0 = hi * 64
                                nc.tensor.matmul(
                                    st[:, hi * 512 : (hi + 1) * 512],
                                    kt[cs][r0 : r0 + 64, ms * 128 : (ms + 1) * 128],
                                    qt[cs][r0 : r0 + 64, n0 : n0 + 512],
                                    start=True,
                                    stop=True,
                                )
                            nc.scalar.activation(
                                out=et[:, mi * 1024 : (mi + 1) * 1024],
                                in_=st,
                                func=mybir.ActivationFunctionType.Exp,
                                scale=float(1.0 / np.sqrt(DK)),
                            )
                            for f in fillers.get(2 * mp + mi, []):
                                f()
                        def do_av(et=et, mp=mp):
                            etv = et.rearrange("p (k n) -> p k n", k=2)
                            v4v = v4p[mp].rearrange("p (k s) -> p k s", k=2)
                            for hi in range(2):
                                h = 2 * cs + hi
                                nc.tensor.matmul(
                                    ot[hi],
                                    v4v[:, :, h * VPITCH : h * VPITCH + 65],
                                    etv[:, :, hi * 512 : (hi + 1) * 512],
                                    start=(mp == 0),
                                    stop=(mp == MP - 1),
                                    perf_mode=mybir.MatmulPerfMode.DoubleRow,
                                )
                        do_av()
                    # normalize: drain ot, broadcast the row-64 denominators
                    # across partitions (GpSimd), reciprocal on the broadcast
                    # (DVE, [64,512] lanes busy), multiply into osb rows.
                    for hi in range(2):
                        oraw = small.tile([65, NCHUNK], F32, tag="oraw", name="oraw_t")
                        nc.vector.tensor_copy(out=oraw, in_=ot[hi])
                        rcin = small.tile([128, NCHUNK // 128], F32, tag="rcin", name="rcin_t")
                        nc.sync.dma_start(out=rcin, in_=oraw[64:65, :])
                        rc = small.tile([128, NCHUNK // 128], F32, tag="rc", name="rc_t")
                        nc.vector.reciprocal(out=rc, in_=rcin)
                        rflat = small.tile([1, NCHUNK], F32, tag="rflat", name="rflat_t")
                        nc.sync.dma_start(out=rflat, in_=rc)
                        rb = small.tile([128, NCHUNK], F32, tag="rb", name="rb_t")
                        nc.gpsimd.partition_broadcast(rb, rflat)
                        nc.vector.tensor_mul(
                            out=osb[(c, cs)][hi * 64 : (hi + 1) * 64, :],
                            in0=oraw[0:64, :],
                            in1=rb[0:64, :],
                        )
                    seg += 1
            # epilogue: only the cs1 half of chunk 3's output projection
            for msl in range(D // 128):
                emit_f3b(msl)
    nc.compile()
    return nc


_NC = None


def _get_nc():
    global _NC
    if _NC is None:
        _NC = _build_bass()
    return _NC


def build_in_maps(inputs):
    x = np.asarray(inputs["x"], dtype=np.float32)
    W_Q = np.asarray(inputs["W_Q"], dtype=np.float32)
    W_K = np.asarray(inputs["W_K"], dtype=np.float32)
    W_V = np.asarray(inputs["W_V"], dtype=np.float32)
    W_O = np.asarray(inputs["W_O"], dtype=np.float32)
    b_Q = np.asarray(inputs["b_Q"], dtype=np.float32)
    b_K = np.asarray(inputs["b_K"], dtype=np.float32)
    b_V = np.asarray(inputs["b_V"], dtype=np.float32)

    in_maps = []
    for core in range(NCORES):
        b, g = divmod(core, GROUPS)
        sl = slice(g * CH, (g + 1) * CH)
        in_maps.append(
            {
                "xT": np.ascontiguousarray(x[b].T.astype(ml_dtypes.bfloat16)),
                "wqT": np.ascontiguousarray(W_Q[sl, :].T.astype(ml_dtypes.bfloat16)),
                "wkT": np.ascontiguousarray(W_K[sl, :].T.astype(ml_dtypes.bfloat16)),
                "wvT": np.ascontiguousarray(W_V[sl, :].T.astype(ml_dtypes.bfloat16)),
                "woT": np.ascontiguousarray(W_O[:, sl].T.astype(ml_dtypes.bfloat16)),
                "bq": np.ascontiguousarray(b_Q[sl]),
                "bk": np.ascontiguousarray(b_K[sl]),
                "bv": np.ascontiguousarray(b_V[sl]),
            }
        )
    return in_maps


def kernel(**inputs):
    in_maps = build_in_maps(inputs)
    nc = _get_nc()
    res = run_bass_kernel_spmd(nc, in_maps, core_ids=list(range(NCORES)))

    b_O = np.asarray(inputs["b_O"], dtype=np.float32)
    out = np.zeros((B, N, D), dtype=np.float32)
    for core in range(NCORES):
        b = core // GROUPS
        out[b] += res.results[core]["yT"].T
    out += b_O
    return out
